# revision 19
# baseline (speedup 1.0000x reference)
"""Trainium2 Bass kernel for the bilinear/demosaic stencil problem.

Full inputs: mosic [16,3,1024,1024] f32, mask [16,3,1024,1024] f32.
Output: clip(mosic + interp*(1-mask), 0, 255)/255, where interp is
  g = g0 + convG(g0)
  r = t + convG(t), t = r0 + convRB(r0)   (same for b)
with convG = cross 3x3 /4, convRB = diagonal 3x3 /4, zero padding.

Sharding: pure data parallel - 2 batch images per core across 8 cores.

This version exploits the loose error tolerance (2e-2) to move all HBM
traffic to uint8 (4x less than f32):

- Host sends mosic rounded to u8 in a padded [H, C*(W+4)] layout (2 zero
  pad columns around each channel row segment, so one DMA descriptor per
  image row covers all channels, pads included - no device memsets).
- The SWDGE (gpsimd) DMA casts u8->fp16 on load; all SBUF data is fp16
  (integers <= 510 are exact in fp16, and all stencil weights are dyadic
  rationals, so the PSUM f32 result v is numerically exact given the u8
  inputs).
- The pre-blend value v = mosic + interp is linear in the input plane X and
  expands over horizontal shifts as in the f32r baseline:
    v_g  = G0 X + GL H1,              G0 = 2I + 0.25V,  GL = 0.25I
    v_rb = A0 X + AL H1 + AV2 (X<LL> + X<RR>) + AVC-edge
           A0 = 2I + 0.375V, AL = 0.25I + 0.25V + 0.0625V^2, AV2 = 0.0625V
  with V the vertical-neighbor band matrix (stationary, fp16) and H1 =
  X<L> + X<R> presummed by one DVE tensor_tensor (fp16, 2x mode).
- Blend + clip: the ACT engine evacuates each 1536-col PSUM tile in one op
  as the REVERSED output w = Relu(255.499 - v) -> fp16 (Relu clamps v>255,
  and w <= 255.5 so the u8 store cast cannot wrap).  The host sends
  Qrev = mask ? 255 - mosic_u8 : 0 (u8, cast-loaded to fp16); since
  v >= mosic always, a single copy_predicated(w <- Qrev where Qrev != 0)
  is exactly the mask blend (when Qrev == 0, either mask == 0, or
  mosic == 255 in which case w is already 0).  The store DMA casts
  fp16 -> u8 and the host returns (255 - out)/255.

HBM traffic per core: 6.3 MB mosic + 6.3 MB Qrev + 6.3 MB out (u8) versus
72 MB for the f32 version.

Images are processed in vertical chunks of 128 input rows with 2-row
overlap; chunk 0 and the last chunk use the true image boundary, which the
finite band matrices handle exactly.
"""

import numpy as np

import concourse.bass as bass
import concourse.bacc as bacc
import concourse.mybir as mybir
import concourse.tile as tile
from concourse.bass_utils import run_bass_kernel_spmd

F32 = mybir.dt.float32
F16 = mybir.dt.float16
I16 = mybir.dt.int16
U8 = mybir.dt.uint8

B, C, H, W = 16, 3, 1024, 1024
N_CORES = 8
BPC = B // N_CORES  # images per core

# matrix slots in the packed weight tensor
G0, GL, A0, AL, AV2, AVC = range(6)

PAD = 2
WB = W + 2 * PAD          # per-channel padded width
FLATW = C * WB            # X tile free size (3084)
CW = C * W                # output tile free size (3072)


def _wmats(P: int) -> np.ndarray:
    """Packed [P, 6*P] stationary matrices (all symmetric, so lhsT == M)."""
    I = np.eye(P, dtype=np.float64)
    V = np.zeros((P, P), np.float64)
    idx = np.arange(P - 1)
    V[idx, idx + 1] = 1.0
    V[idx + 1, idx] = 1.0
    V2 = V @ V
    mats = [
        2 * I + 0.25 * V,                    # G0
        0.25 * I,                            # GL
        2 * I + 0.375 * V,                   # A0
        0.25 * I + 0.25 * V + 0.0625 * V2,   # AL
        0.0625 * V,                          # AV2
        -0.0625 * V,                         # AVC (edge-column correction)
    ]
    return np.concatenate(mats, axis=1).astype(np.float16)


def _chunks():
    """(in_row_start a, in_rows P, out_row_start o, out_rows OR, valid_off vo)."""
    out = [(0, 128, 0, 126, 0)]
    o = 126
    while o + 124 <= H - 30:
        out.append((o - 2, 128, o, 124, 2))
        o += 124
    a = H - 32
    out.append((a, 32, o, H - o, o - a))
    return out


def _build_nc():
    nc = bacc.Bacc(trn_type="TRN2")
    mos = nc.dram_tensor("mosic", [BPC, H, FLATW], U8, kind="ExternalInput")
    qrev = nc.dram_tensor("qrev", [BPC, H, CW], U8, kind="ExternalInput")
    w128 = nc.dram_tensor("w128", [128, 6 * 128], F16, kind="ExternalInput")
    w32 = nc.dram_tensor("w32", [32, 6 * 32], F16, kind="ExternalInput")
    out = nc.dram_tensor("out", [BPC, H, CW], U8, kind="ExternalOutput")

    with tile.TileContext(nc) as tc:
        with (
            tc.tile_pool(name="wp", bufs=1) as wp,
            tc.tile_pool(name="xp", bufs=4) as xp,
            tc.tile_pool(name="qp", bufs=4) as qp,
            tc.tile_pool(name="h1p", bufs=4) as h1p,
            tc.tile_pool(name="wop", bufs=3) as wop,
            tc.tile_pool(name="psp", bufs=4, space="PSUM") as psp,
        ):
            chunks_all = [(img, ch) for img in range(BPC) for ch in _chunks()]
            NCH = len(chunks_all)
            PF = 2  # load prefetch depth (chunks)

            def load_X(k):
                img, (a, P, o, OR, vo) = chunks_all[k]
                X = xp.tile([128, FLATW], F16, tag="X", name=f"X{k}")
                nc.gpsimd.dma_start(X[0:P], mos[img][a:a + P, :])
                return X

            def load_Q(k):
                img, (a, P, o, OR, vo) = chunks_all[k]
                Q = qp.tile([128, CW], U8, tag="Q", name=f"Q{k}")
                nc.sync.dma_start(Q[0:P], qrev[img][a:a + P, :])
                return Q

            xtiles = {k: load_X(k) for k in range(PF)}
            qtiles = {k: load_Q(k) for k in range(PF)}

            wt128 = wp.tile([128, 6 * 128], F16)
            nc.sync.dma_start(wt128[:], w128[:])
            wt32 = wp.tile([32, 6 * 32], F16)
            nc.sync.dma_start(wt32[:], w32[:])
            b255 = wp.tile([128, 1], F32)
            nc.gpsimd.memset(b255[:], 255.499)
            # warm the ACT Relu table while the first loads are in flight
            warm = wp.tile([128, 1], F16)
            nc.scalar.activation(
                warm[:], b255[:], mybir.ActivationFunctionType.Relu,
                bias=b255[0:128, 0:1], scale=-1.0,
            )

            pending_store = []

            def flush_store(keep=0):
                while len(pending_store) > keep:
                    Ws, simg, so, sOR, svo = pending_store.pop(0)
                    cuts = [svo] + [p for p in (32, 64, 96) if svo < p < svo + sOR] \
                        + [svo + sOR]
                    for sv, sv1 in zip(cuts, cuts[1:]):
                        r0 = so + (sv - svo)
                        nc.sync.dma_start(
                            out[simg][r0:r0 + (sv1 - sv), :],
                            Ws[sv:sv1],
                        )

            def compute_H1(k):
                img, (a, P, o, OR, vo) = chunks_all[k]
                X = xtiles[k]
                H1 = h1p.tile([128, FLATW - 2], F16, tag="H1", name=f"H1_{k}")
                nc.vector.tensor_tensor(
                    H1[0:P], X[0:P, 0:FLATW - 2], X[0:P, 2:FLATW],
                    mybir.AluOpType.add,
                )
                return H1

            h1tiles = {k: compute_H1(k) for k in range(PF)}

            for ci in range(NCH):
                img, (a, P, o, OR, vo) = chunks_all[ci]
                flush_store(keep=0)
                if ci + PF < NCH:
                    xtiles[ci + PF] = load_X(ci + PF)
                    qtiles[ci + PF] = load_Q(ci + PF)
                    # H1 two chunks ahead keeps the GL/AL matmul deps clear
                    # of this chunk's blend in the DVE stream.
                    h1tiles[ci + PF] = compute_H1(ci + PF)
                X = xtiles.pop(ci)
                Q = qtiles.pop(ci)
                H1 = h1tiles.pop(ci)
                wt = wt128 if P == 128 else wt32

                def lhs(k):
                    return wt[0:P, k * P:(k + 1) * P]

                # per-channel psum tiles; alloc order (c1, c0, c2) matches
                # evac order so the 4-slot rotation always reuses the
                # earliest-freed bank pair.
                ps1 = psp.tile([128, 1024], F32, tag="ps", name=f"ps1_{ci}")
                ps0 = psp.tile([128, 1024], F32, tag="ps", name=f"ps0_{ci}")
                ps2 = psp.tile([128, 1024], F32, tag="ps", name=f"ps2_{ci}")
                pst = {0: ps0, 1: ps1, 2: ps2}

                def pslice(c, h, col=None, n=512):
                    f0 = h * 512 + (col or 0)
                    return pst[c][0:P, f0:f0 + n]

                def xsl(c, h, d=0, n=512):
                    f = c * WB + PAD + h * 512 + d
                    return X[0:P, f:f + n]

                def h1sl(c, h):
                    f = c * WB + 1 + h * 512
                    return H1[0:P, f:f + 512]

                Wt = wop.tile([128, CW], U8, tag="Wt", name=f"W{ci}")
                last = ci == NCH - 1

                def evac(c):
                    nc.scalar.activation(
                        Wt[0:P, c * 1024:(c + 1) * 1024], pst[c][0:P, :],
                        mybir.ActivationFunctionType.Relu,
                        bias=b255[0:P, 0:1], scale=-1.0,
                    )
                    if last:
                        # shorten the tail: blend each channel right after
                        # its evac instead of one wide op at the end
                        nc.vector.tensor_tensor(
                            Wt[0:P, c * 1024:(c + 1) * 1024],
                            Wt[0:P, c * 1024:(c + 1) * 1024],
                            Q[0:P, c * 1024:(c + 1) * 1024],
                            mybir.AluOpType.max,
                        )

                # G channel first (2 matrices) so its psum frees earliest.
                for h in range(2):
                    nc.tensor.matmul(pslice(1, h), lhs(G0), xsl(1, h),
                                     start=True, stop=False)
                for h in range(2):
                    nc.tensor.matmul(pslice(1, h), lhs(GL), h1sl(1, h),
                                     start=False, stop=True)
                evac(1)
                for c in (0, 2):
                    for h in range(2):
                        nc.tensor.matmul(pslice(c, h), lhs(A0), xsl(c, h),
                                         start=True, stop=False)
                for c in (0, 2):
                    for h in range(2):
                        nc.tensor.matmul(pslice(c, h), lhs(AL), h1sl(c, h),
                                         start=False, stop=False)
                for d in (2, -2):
                    for c in (0, 2):
                        for h in range(2):
                            nc.tensor.matmul(pslice(c, h), lhs(AV2),
                                             xsl(c, h, d),
                                             start=False, stop=False)
                # edge correction: the L/R expansion over-counts V at the
                # image's first/last column; subtract 0.0625*V there.
                for c in (0, 2):
                    for h in range(2):
                        ecol = 0 if h == 0 else W - 1
                        ocol = 0 if h == 0 else 511
                        nc.tensor.matmul(
                            pslice(c, h, col=ocol, n=1),
                            lhs(AVC),
                            X[0:P, c * WB + PAD + ecol:c * WB + PAD + ecol + 1],
                            start=False, stop=True,
                        )
                    evac(c)

                # mask blend: max(w, Qrev) equals the predicated overwrite
                # because v >= mosic everywhere.
                if not last:
                    nc.vector.tensor_tensor(
                        Wt[0:P], Wt[0:P], Q[0:P], mybir.AluOpType.max,
                    )

                pending_store.append((Wt, img, o, OR, vo))

            flush_store()

    nc.finalize()
    return nc


_CACHE: dict = {}


def _get_nc():
    if "nc" not in _CACHE:
        _CACHE["nc"] = _build_nc()
    return _CACHE["nc"]


def _prep_inputs(mosic, mask):
    mosic = np.asarray(mosic, dtype=np.float32)
    mask = np.asarray(mask, dtype=np.float32)
    m8 = np.rint(np.clip(mosic, 0.0, 255.0)).astype(np.uint8)  # [B,C,H,W]
    m8t = m8.transpose(0, 2, 1, 3)                             # [B,H,C,W]
    mos_p = np.zeros((B, H, C, W + 2 * PAD), np.uint8)
    mos_p[:, :, :, PAD:PAD + W] = m8t
    mos_p = mos_p.reshape(B, H, FLATW)
    q = np.where(mask != 0.0, 255 - m8, 0).astype(np.uint8)    # [B,C,H,W]
    qrev = np.ascontiguousarray(q.transpose(0, 2, 1, 3)).reshape(B, H, CW)
    return mos_p, qrev


def _run(mosic, mask, **spmd_kwargs):
    spmd_kwargs.pop("mm_dt", None)
    nc = _get_nc()
    mos_p, qrev = _prep_inputs(mosic, mask)
    w128 = _wmats(128)
    w32 = _wmats(32)
    in_maps = []
    for cid in range(N_CORES):
        sl = slice(cid * BPC, (cid + 1) * BPC)
        in_maps.append({
            "mosic": mos_p[sl],
            "qrev": qrev[sl],
            "w128": w128,
            "w32": w32,
        })
    res = run_bass_kernel_spmd(nc, in_maps, core_ids=list(range(N_CORES)), **spmd_kwargs)
    out_u8 = np.concatenate([r["out"] for r in res.results], axis=0)  # [B,H,CW]
    out_u8 = out_u8.reshape(B, H, C, W).transpose(0, 2, 1, 3)         # [B,C,H,W]
    full = (np.float32(255.0) - out_u8.astype(np.float32)) * np.float32(1.0 / 255.0)
    return full, res


def kernel(mosic, mask):
    full, _ = _run(mosic, mask)
    return full
